# revision 15
# baseline (speedup 1.0000x reference)
"""Trainium2 Bass kernel for nn_AttentionLayer (4x2048x768, d_k=128, d_v=768).

Sharding (sequence-parallel over keys, data-parallel over batch):
8 cores; core c handles batch b=c//2 with KEY half h=c%2. Each core computes
q for ALL 2048 queries but k/v only for its own 1024 keys, then produces the
partial (unnormalized) attention numerator plus the partial softmax row sum.

fp8 DoubleRow acceleration (2x PE throughput, 256-deep contraction/inst) for
the two dominant matmuls (v-projection and attn@V numerator), with two error
mitigations that keep rel err ~1.4e-2 (< 2e-2 gate):

  1. expm1 trick: the matmul uses P' = exp(s) - 1 quantized to fp8e4 instead
     of exp(s). Softmax weights here are ~1 +- 0.35, so |P'| << |P| and the
     fp8 quantization error shrinks ~3x. The dropped "1" contributes
     colsum_v[n] = sum_t v[t,n] to every query's numerator and T to every
     row sum; both are restored EXACTLY on the host:
         out = (colsum_v + sum_cores P'8@v8/32) / (2048 + sum_cores P'8@1)
     with colsum_v = (sum_t x_t) @ Wv computed in f64 (tiny: 768x768).
  2. The same colsum restore also cancels the common-mode (p-bar-weighted)
     component of the v-side fp8 quantization error.

Numerics per core:
  q/k proj + scores: bf16 (score accuracy dominates overall error).
  exp -> P_hi bf16 (scalar ACT engine), then the vector engine computes
  P'8 = fp8(P_hi - 1) (gpsimd is ~15x too slow for this).
  v-proj: x8 fp8 x wv8 (=32*Wv in fp8) DoubleRow -> psum = 32*v ->
  v8 = fp8(32*v) (vector cast). The 32x pre-scale keeps Wv's tiny uniform
  (+-0.036) values out of fp8e4's subnormal range; host divides by 32.
  x8 is cast from bf16 x on-chip (vector, idle during q/k) - no extra DMA.
  numerator: P'8 x v8 DoubleRow, t-chunk pairs packed in the two slots.
  bk is dropped (softmax-invariant); bq/bv handled as in the baseline.

Timing model (measured): the DMA queues cannot issue until ~+7us of
framework preamble and deliver ~0.11MB/us EACH, so inputs are spread over
FOUR queues (scalar/sync/gpsimd/vector) with WAW gates serializing each
queue's loads in consumption order. Data-free warm-up matmuls burn the
~6us PE p-state ramp during the preamble window. Output stores alternate
the scalar/sync rings; the last two query tiles store solo/split so the
post-matmul drain is short.
"""

import sys

sys.path.insert(0, "/opt/trn_rl_repo")

import numpy as np
import ml_dtypes

B, T, DIN, DK, DV = 4, 2048, 768, 128, 768
NCORES = 8
TOWN = 1024  # own keys per core
CH = DIN // 128  # 6 contraction chunks over d_in
TCH = TOWN // 128  # 8 own-key chunks
QCH = T // 128  # 16 query chunks (all queries)
SCALE = 1.0 / float(np.sqrt(DK))
VSCALE = 32.0  # fp8 pre-scale on Wv (power of 2; host divides out)

_CACHE = {}


def _build():
    from contextlib import ExitStack

    from concourse import bacc, mybir, tile

    f32 = mybir.dt.float32
    bf16 = mybir.dt.bfloat16
    fp8 = mybir.dt.float8e4
    DR = mybir.MatmulPerfMode.DoubleRow

    nc = bacc.Bacc("TRN2", target_bir_lowering=False, debug=False)

    x_own = nc.dram_tensor("x_own", [128, 3, 2 * TOWN], bf16, kind="ExternalInput").ap()
    x_oth = nc.dram_tensor("x_oth", [128, CH, TOWN], bf16, kind="ExternalInput").ap()
    wq = nc.dram_tensor("wq", [128, CH, DK], bf16, kind="ExternalInput").ap()
    wk = nc.dram_tensor("wk", [128, CH, DK], bf16, kind="ExternalInput").ap()
    wv8 = nc.dram_tensor("wv8", [128, CH, DV], fp8, kind="ExternalInput").ap()
    bq = nc.dram_tensor("bq", [DK, 1], f32, kind="ExternalInput").ap()
    out = nc.dram_tensor("out", [128, QCH, DV + 1], bf16, kind="ExternalOutput").ap()

    with tile.TileContext(nc) as tc, ExitStack() as ctx:
        consts = ctx.enter_context(tc.tile_pool(name="consts", bufs=1))
        persist = ctx.enter_context(tc.tile_pool(name="persist", bufs=1))
        wpool = ctx.enter_context(tc.tile_pool(name="wpool", bufs=1))
        xpool = ctx.enter_context(tc.tile_pool(name="xpool", bufs=1))
        ph_pool = ctx.enter_context(tc.tile_pool(name="ph", bufs=3))
        out_pool = ctx.enter_context(tc.tile_pool(name="out_pool", bufs=4))
        ps_pool = ctx.enter_context(tc.tile_pool(name="ps", bufs=2, space="PSUM"))
        sc_pool = ctx.enter_context(tc.tile_pool(name="sc", bufs=2, space="PSUM"))

        # --- warm-up first: memset leads the gpsimd queue so the data-free
        # matmuls can start burning the PE p-state ramp at ~+2.5us.
        warm = consts.tile([128, 640], bf16)
        nc.gpsimd.memset(warm[:], 0.0)
        for _ in range(15):
            ps_w = ps_pool.tile([128, 1024], f32, tag="ps")
            nc.tensor.matmul(
                ps_w[:, 0:512], warm[:, 0:128], warm[:, 128:640], start=True, stop=True
            )

        qT_sb = persist.tile([128, T], bf16)  # [dk, q] all queries, q̂+bq
        kT_sb = persist.tile([128, TOWN], bf16)  # [dk, t-own]
        v8_sb = persist.tile([128, TCH, DV + 2], fp8)  # [t-part, chunk, 32v|1|0]
        pT8_sb = persist.tile([128, TCH, T], fp8)  # [t-part, chunk, q] = exp-1
        bq_sb = consts.tile([DK, 1], f32)

        xo_sb = xpool.tile([128, 3, 2 * TOWN], bf16)  # pair p: chunks 2p|2p+1
        xt_sb = xpool.tile([128, CH, TOWN], bf16)
        x8o_sb = xpool.tile([128, 3, 2, TOWN], fp8)
        wq_sb = wpool.tile([128, CH, DK], bf16)
        wk_sb = wpool.tile([128, CH, DK], bf16)
        wv8_sb = wpool.tile([128, CH, DV], fp8)

        def xo(c):  # own-x chunk c -> [128, TOWN] slice of the pair tile
            return xo_sb[:, c // 2, (c % 2) * TOWN : (c % 2 + 1) * TOWN]

        # --- input DMAs over FOUR queues, x pairs land in q/k consumption
        # order. WAW gates (1-element copies) serialize each queue's loads:
        # a ring round-robins over all queued DMAs, so an ungated late load
        # steals bandwidth from the critical early ones.
        nc.gpsimd.dma_start(out=xo_sb[:, 0, :], in_=x_own[:, 0, :])
        nc.gpsimd.dma_start(out=bq_sb[:], in_=bq)
        nc.scalar.dma_start(out=wq_sb[:], in_=wq)
        nc.sync.dma_start(out=xo_sb[:, 1, :], in_=x_own[:, 1, :])
        # gates + held-back loads
        nc.vector.tensor_copy(xo_sb[:, 2, 0:1], wq_sb[:, 0, 0:1])
        nc.scalar.dma_start(out=xo_sb[:, 2, :], in_=x_own[:, 2, :])
        nc.vector.tensor_copy(wk_sb[:, 0, 0:1], xo_sb[:, 1, 0:1])
        nc.sync.dma_start(out=wk_sb[:], in_=wk)
        nc.vector.tensor_copy(wv8_sb[:, 0, 0:1], xo_sb[:, 0, 0:1])
        nc.gpsimd.dma_start(out=wv8_sb[:], in_=wv8)

        nc.vector.memset(v8_sb[:, :, DV : DV + 1], 1.0)
        nc.vector.memset(v8_sb[:, :, DV + 1 : DV + 2], 0.0)
        # on-chip bf16 -> fp8 casts of own-x (vector is idle during q/k);
        # emitted BEFORE the x-oth gates so those (blocked on later pairs)
        # don't head-of-line-block the casts in the vector queue.
        for p in range(3):
            for s in range(2):
                nc.vector.tensor_copy(
                    x8o_sb[:, p, s, :], xo_sb[:, p, s * TOWN : (s + 1) * TOWN]
                )

        nc.vector.tensor_copy(xt_sb[:, 0, 0:1], wk_sb[:, 0, 1:2])
        nc.sync.dma_start(out=xt_sb[:, 0:3, :], in_=x_oth[:, 0:3, :])
        nc.vector.tensor_copy(xt_sb[:, 3, 0:1], xo_sb[:, 2, 1:2])
        nc.scalar.dma_start(out=xt_sb[:, 3:6, :], in_=x_oth[:, 3:6, :])

        def emit_scores_t(t, qh):
            # scores^T for one own-key chunk -> P'8 = fp8(exp(scale*s) - 1).
            # Two 512-col matmuls fill a 2-bank psum tile; ONE 1024-col exp
            # and ONE 1024-col vector (-1 + fp8 cast) consume it.
            ps_s = sc_pool.tile([128, 1024], f32, tag="sc")
            for n0 in (0, 512):
                nc.tensor.matmul(
                    ps_s[:, n0 : n0 + 512],
                    kT_sb[:, t * 128 : (t + 1) * 128],
                    qT_sb[:, qh * 1024 + n0 : qh * 1024 + n0 + 512],
                    start=True,
                    stop=True,
                )
            ph = ph_pool.tile([128, 1024], bf16, tag="ph")
            nc.scalar.activation(
                ph[:], ps_s[:], mybir.ActivationFunctionType.Exp, scale=SCALE
            )
            nc.vector.tensor_scalar(
                out=pT8_sb[:, t, qh * 1024 : qh * 1024 + 1024],
                in0=ph[:],
                scalar1=1.0,
                scalar2=None,
                op0=mybir.AluOpType.subtract,
            )

        def emit_v_t(t):
            # v-projection for one own-key chunk: fp8 DoubleRow over d_in
            # pairs; psum accumulates 32*v; cast to fp8 keeps the 32x scale.
            ps_v = ps_pool.tile([128, 1024], f32, tag="ps")
            for p in range(3):
                for n0, n1 in ((0, 512), (512, DV)):
                    nc.tensor.matmul(
                        ps_v[:, n0:n1],
                        x8o_sb[:, p, :, t * 128 : (t + 1) * 128],
                        wv8_sb[:, 2 * p : 2 * p + 2, n0:n1],
                        start=(p == 0),
                        stop=(p == 2),
                        perf_mode=DR,
                    )
            nc.vector.tensor_copy(v8_sb[:, t, 0:DV], ps_v[:, 0:DV])

        # qc pairs share one SBUF tile and one store DMA; the last two tiles
        # store solo/split so the drain tail is short.
        o_state = {}

        def emit_out_qc(qc):
            # partial numerator + rowsum: out[qc] = sum_t P'8[t,qc].T @ [32v|1]
            # fp8 DoubleRow, t-chunk pairs in the two slots.
            ps_o = ps_pool.tile([128, 1024], f32, tag="ps")
            if qc % 2 == 0:
                o_pair = out_pool.tile([128, 2, DV + 1], bf16, tag="o")
                o_state["tile"] = o_pair
            o_sb = o_state["tile"][:, qc % 2, :]
            for reg, (n0, n1) in enumerate(((0, 512), (512, DV + 2))):
                for tp in range(4):
                    nc.tensor.matmul(
                        ps_o[:, n0:n1],
                        pT8_sb[:, 2 * tp : 2 * tp + 2, qc * 128 : (qc + 1) * 128],
                        v8_sb[:, 2 * tp : 2 * tp + 2, n0:n1],
                        start=(tp == 0),
                        stop=(tp == 3),
                        perf_mode=DR,
                    )
                c1 = min(n1, DV + 1)
                nc.vector.tensor_copy(o_sb[:, n0:c1], ps_o[:, n0:c1])
                if qc >= QCH - 2:
                    # last two tiles: store each region immediately, split
                    # by partition across both rings
                    nc.sync.dma_start(
                        out=out[0:64, qc, n0:c1], in_=o_sb[0:64, n0:c1]
                    )
                    nc.scalar.dma_start(
                        out=out[64:128, qc, n0:c1], in_=o_sb[64:128, n0:c1]
                    )
                elif qc % 2 == 1 and reg == 1:
                    # pair complete: one contiguous 2-tile store
                    eng = nc.sync if (qc // 2) % 2 == 0 else nc.scalar
                    eng.dma_start(
                        out=out[:, qc - 1 : qc + 1, :], in_=o_state["tile"][:]
                    )

        # q own-half then k own, each a single run of region-alternating mms
        # into ONE psum tile (psum switches cost a PE pipeline flush).
        # Chunk order matches DMA arrival order (pair0, pair1, pair2).
        ps_q0 = ps_pool.tile([128, 1024], f32, tag="ps")
        ps_k = ps_pool.tile([128, 1024], f32, tag="ps")
        for dst, w_sb in ((ps_q0, wq_sb), (ps_k, wk_sb)):
            for c in range(CH):
                for n0 in (0, 512):
                    nc.tensor.matmul(
                        dst[:, n0 : n0 + 512],
                        w_sb[:, c, :],
                        xo(c)[:, n0 : n0 + 512],
                        start=(c == 0),
                        stop=(c == CH - 1),
                    )
        # qT = q̂+bq on scalar; kT single bulk cast on vector
        nc.scalar.activation(
            qT_sb[:, 0:TOWN],
            ps_q0[:],
            mybir.ActivationFunctionType.Identity,
            bias=bq_sb[:],
        )
        nc.vector.tensor_copy(kT_sb[:], ps_k[:])

        # scores for own queries interleaved with v-projection
        for t in range(TCH):
            emit_scores_t(t, 0)
            emit_v_t(t)

        # q other-half
        ps_q1 = ps_pool.tile([128, 1024], f32, tag="ps")
        for c in range(CH):
            for n0 in (0, 512):
                nc.tensor.matmul(
                    ps_q1[:, n0 : n0 + 512],
                    wq_sb[:, c, :],
                    xt_sb[:, c, n0 : n0 + 512],
                    start=(c == 0),
                    stop=(c == CH - 1),
                )
        nc.scalar.activation(
            qT_sb[:, TOWN : 2 * TOWN],
            ps_q1[:],
            mybir.ActivationFunctionType.Identity,
            bias=bq_sb[:],
        )

        # scores for other-half queries interleaved with out
        for qc in range(8):
            emit_scores_t(qc, 1)
            emit_out_qc(qc)

        for qc in range(8, 16):
            emit_out_qc(qc)

    nc.compile()
    return nc


def _get_nc():
    if "nc" not in _CACHE:
        _CACHE["nc"] = _build()
    return _CACHE["nc"]


def _make_in_maps(x, Wq, bq, Wk, bk, Wv):
    bf16 = ml_dtypes.bfloat16
    fp8 = ml_dtypes.float8_e4m3
    base = {
        "wq": np.ascontiguousarray(
            np.asarray(Wq, np.float32).astype(bf16).reshape(CH, 128, DK).transpose(1, 0, 2)
        ),
        "wk": np.ascontiguousarray(
            np.asarray(Wk, np.float32).astype(bf16).reshape(CH, 128, DK).transpose(1, 0, 2)
        ),
        "wv8": np.ascontiguousarray(
            (np.asarray(Wv, np.float32) * VSCALE)
            .astype(fp8)
            .reshape(CH, 128, DV)
            .transpose(1, 0, 2)
        ),
        "bq": np.ascontiguousarray(np.asarray(bq, np.float32).reshape(DK, 1)),
    }
    in_maps = []
    for c in range(NCORES):
        b, h = c // 2, c % 2
        xb = x[b]  # [T, DIN]
        rot = np.concatenate([xb[h * TOWN :], xb[: h * TOWN]], axis=0)
        xT = rot.T.astype(bf16).reshape(CH, 128, T).transpose(1, 0, 2)  # [128,c,t]
        own = xT[:, :, 0:TOWN]  # [128, c, 1024]
        m = dict(base)
        m["x_own"] = np.ascontiguousarray(own.reshape(128, 3, 2 * TOWN))
        m["x_oth"] = np.ascontiguousarray(xT[:, :, TOWN:T])
        in_maps.append(m)
    return in_maps


def kernel(x, Wq, bq, Wk, bk, Wv, bv):
    from concourse import bass_utils

    x = np.ascontiguousarray(np.asarray(x, dtype=np.float32))
    nc = _get_nc()
    in_maps = _make_in_maps(x, Wq, bq, Wk, bk, Wv)

    res = bass_utils.run_bass_kernel_spmd(nc, in_maps, core_ids=list(range(NCORES)))

    x64 = np.asarray(x, np.float64)
    Wv64 = np.asarray(Wv, np.float64)
    bv64 = np.asarray(bv, np.float64).reshape(1, DV)
    outp = np.empty((B, T, DV), dtype=np.float32)
    for b in range(B):
        # out is partition-major [128, qc, 769] -> [qc*128+p, 769]
        p0 = res.results[2 * b]["out"].transpose(1, 0, 2).reshape(T, DV + 1)
        p1 = res.results[2 * b + 1]["out"].transpose(1, 0, 2).reshape(T, DV + 1)
        p1 = np.concatenate([p1[TOWN:], p1[:TOWN]], axis=0)
        s = p0.astype(np.float64) + p1.astype(np.float64)
        colsum = x64[b].sum(axis=0) @ Wv64  # exact f64 restore of the
        num = s[:, 0:DV] / VSCALE + colsum[None, :]  # dropped "+1" in expm1
        den = s[:, DV : DV + 1] + float(T)
        outp[b] = (num / den + bv64).astype(np.float32)
    return outp
